# revision 41
# baseline (speedup 1.0000x reference)
"""Distributed GQA attention kernel for 8 TRN2 NeuronCores.

Problem: B=1, S=2048, D=4096, H=32 q-heads, KV=8 kv-heads, HD=128.
  q = rope(x@wq.T), k = rope(x@wk.T), v = x@wv.T
  out = softmax(causal(q@k.T/sqrt(HD))) @ v @ wo.T

Sharding: tensor-parallel over heads. Core c owns q-heads 4c..4c+3 and
kv-head c. Device-side per core:
  phase 1: QT/KT (rope'd, [hd, s] layout) + VT projections; V tiles
           ([t, hd]) via DMA transpose
  phase 2: causal attention producing attnT chunks; software-pipelined:
           epilogues deferred one head, AllGather + out-proj deferred
           one s-chunk so they overlap the next chunk's attention.
Host side: layout prep (transposes, bf16 cast, rope tables) + final
concat/transpose of the 8 out.T slices.
"""

import math
import numpy as np
import ml_dtypes

BF = ml_dtypes.bfloat16

B, S, D = 1, 2048, 4096
H, KV, HD = 32, 8, 128
NCORES = 8
HL = H // NCORES            # 4 local q heads
QW = HL * HD                # 512 local q width
SC = 512                    # s-chunk width
NSC = S // SC               # 4 s-chunks
KD = 32                     # d-dim k-tiles (4096/128)
NT = S // 128               # 16 t-tiles
SCALE = 1.0 / math.sqrt(HD)
NEG = -30000.0


def _build_nc():
    import concourse.bass as bass
    import concourse.mybir as mybir
    from concourse import bacc, tile

    dt = mybir.dt
    nc = bacc.Bacc()

    xt_d = nc.declare_dram_parameter("xt", [D, S], dt.bfloat16, isOutput=False)
    wqt_d = nc.declare_dram_parameter("wqt", [D, QW], dt.bfloat16, isOutput=False)
    wkt_d = nc.declare_dram_parameter("wkt", [D, HD], dt.bfloat16, isOutput=False)
    wvt_d = nc.declare_dram_parameter("wvt", [D, HD], dt.bfloat16, isOutput=False)
    wot_d = nc.declare_dram_parameter("wot", [D, QW], dt.bfloat16, isOutput=False)
    cosd_d = nc.declare_dram_parameter("cosd", [HD, S], dt.bfloat16, isOutput=False)
    sind_d = nc.declare_dram_parameter("sind", [HD, S], dt.bfloat16, isOutput=False)
    swapt_d = nc.declare_dram_parameter("swapt", [HD, HD], dt.bfloat16, isOutput=False)
    ident_d = nc.declare_dram_parameter("ident", [HD, HD], dt.bfloat16, isOutput=False)
    dmask_d = nc.declare_dram_parameter("dmask", [128, 128], dt.float32, isOutput=False)
    onesc_d = nc.declare_dram_parameter("onesc", [128, 1], dt.bfloat16, isOutput=False)
    onesr_d = nc.declare_dram_parameter("onesr", [1, 128], dt.float32, isOutput=False)
    out_d = nc.declare_dram_parameter("out_t", [QW, S], dt.float32, isOutput=True)

    with tile.TileContext(nc) as tc:
        with (
            tc.tile_pool(name="const", bufs=1) as cpool,
            tc.tile_pool(name="qkv", bufs=1) as qkvpool,
            tc.tile_pool(name="att", bufs=1) as attpool,
            tc.tile_pool(name="dram", bufs=1, space="DRAM") as dpool,
        ):
            # ---- small resident constants ----
            cosd = cpool.tile([HD, S], dt.bfloat16)
            sind = cpool.tile([HD, S], dt.bfloat16)
            swapt = cpool.tile([HD, HD], dt.bfloat16)
            ident = cpool.tile([HD, HD], dt.bfloat16)
            dmask = cpool.tile([128, 128], dt.float32)
            onesc = cpool.tile([128, 1], dt.bfloat16)
            onesr = cpool.tile([1, 128], dt.float32)
            nc.sync.dma_start(cosd[:], cosd_d[:, :])
            nc.sync.dma_start(sind[:], sind_d[:, :])
            nc.sync.dma_start(swapt[:], swapt_d[:, :])
            nc.sync.dma_start(ident[:], ident_d[:, :])
            nc.sync.dma_start(dmask[:], dmask_d[:, :])
            nc.sync.dma_start(onesc[:], onesc_d[:, :])
            nc.sync.dma_start(onesr[:], onesr_d[:, :])
            # warm up the ACT exp table load before attention needs it
            warm = cpool.tile([1, 1], dt.float32)
            nc.scalar.activation(warm[:], onesr[0:1, 0:1],
                                 mybir.ActivationFunctionType.Exp)

            # ---- persistent activations ----
            qt = [qkvpool.tile([HD, S], dt.bfloat16, name=f"qt{h}", tag=f"qt{h}")
                  for h in range(HL)]
            kt = qkvpool.tile([HD, S], dt.bfloat16)
            vv = qkvpool.tile([128, NT, HD], dt.bfloat16)   # [t_part, ti, hd]
            att = [attpool.tile([HD, S], dt.bfloat16, name=f"att{h}", tag=f"att{h}")
                   for h in range(HL)]

            xt_r = xt_d[:, :].rearrange("(k p) s -> p k s", p=128)

            # ================= phase 1: projections + rope =================
            with (
                tc.tile_pool(name="w1", bufs=1) as wpool,
                tc.tile_pool(name="xc", bufs=2) as xpool,
                tc.tile_pool(name="p1", bufs=3, space="PSUM") as pp1,
                tc.tile_pool(name="pr", bufs=2, space="PSUM") as ppr,
                tc.tile_pool(name="rtmp", bufs=3) as rtpool,
            ):
                wqt = wpool.tile([128, KD, QW], dt.bfloat16)
                wkt = wpool.tile([128, KD, HD], dt.bfloat16)
                wvt = wpool.tile([128, KD, HD], dt.bfloat16)
                vt = wpool.tile([HD, S], dt.bfloat16)
                wqt_r = wqt_d[:, :].rearrange("(k p) n -> p k n", p=128)
                # interleave first x-chunk with weight loads so the first
                # matmuls can start as early as possible
                xc0 = xpool.tile([128, KD, SC], dt.bfloat16, tag="xc")
                pieces = [(0, 2), (2, 4), (4, 8), (8, 12), (12, 16),
                          (16, 20), (20, 24), (24, 28), (28, 32)]
                for lo, hi in pieces:
                    ksl = slice(lo, hi)
                    nc.sync.dma_start(xc0[:, ksl, :], xt_r[:, ksl, 0:SC])
                    nc.sync.dma_start(wqt[:, ksl, :], wqt_r[:, ksl, :])
                nc.sync.dma_start(
                    wkt[:], wkt_d[:, :].rearrange("(k p) n -> p k n", p=128))
                nc.sync.dma_start(
                    wvt[:], wvt_d[:, :].rearrange("(k p) n -> p k n", p=128))

                # tiny dummy collective to absorb first-use RDH/ncfw setup
                # cost while the projections run
                cw_in = dpool.tile([128, 16], dt.bfloat16)
                cw_out = dpool.tile([NCORES * 128, 16], dt.bfloat16,
                                    addr_space="Shared")
                nc.sync.dma_start(cw_in[:], xt_d[0:128, 0:16])
                nc.gpsimd.collective_compute(
                    "AllGather",
                    mybir.AluOpType.bypass,
                    replica_groups=[list(range(NCORES))],
                    ins=[cw_in.opt()],
                    outs=[cw_out.opt()],
                )

                for sc in range(NSC):
                    ssl = slice(sc * SC, (sc + 1) * SC)
                    if sc == 0:
                        xc = xc0
                    else:
                        xc = xpool.tile([128, KD, SC], dt.bfloat16, tag="xc")
                        for kg in range(4):
                            ksl = slice(kg * 8, (kg + 1) * 8)
                            nc.sync.dma_start(xc[:, ksl, :], xt_r[:, ksl, ssl])

                    # 4 Q heads (rope), K (rope), V (plain) — all [hd, s]
                    for hi in range(HL + 2):
                        ps = pp1.tile([128, SC], dt.float32)
                        for k in range(KD):
                            if hi < HL:
                                lhs = wqt[:, k, hi * HD:(hi + 1) * HD]
                            elif hi == HL:
                                lhs = wkt[:, k, :]
                            else:
                                lhs = wvt[:, k, :]
                            nc.tensor.matmul(ps[:], lhs, xc[:, k, :],
                                             start=(k == 0), stop=(k == KD - 1))
                        if hi == HL + 1:
                            nc.scalar.copy(vt[:, ssl], ps[:])
                            continue
                        # rope: out = q*cos + rot(q)*sin, rot via swap-matmul
                        qs = rtpool.tile([128, SC], dt.bfloat16, tag="ropeqs")
                        qc = rtpool.tile([128, SC], dt.bfloat16, tag="ropeqc")
                        nc.vector.tensor_mul(qs[:], ps[:], sind[:, ssl])
                        nc.vector.tensor_mul(qc[:], ps[:], cosd[:, ssl])
                        ps2 = ppr.tile([128, SC], dt.float32)
                        nc.tensor.matmul(ps2[:], swapt[:], qs[:], start=True, stop=False)
                        nc.tensor.matmul(ps2[:], ident[:], qc[:], start=False, stop=True)
                        dst = qt[hi] if hi < HL else kt
                        nc.scalar.copy(dst[:, ssl], ps2[:])

                    # V tiles in [t, hd] layout via DMA transpose
                    for vtile in range(4):
                        ti = sc * 4 + vtile
                        nc.sync.dma_start_transpose(
                            vv[:, ti, :], vt[:, ti * 128:(ti + 1) * 128])

            # ============ phase 2+3: attention, allgather, out-proj ============
            with (
                tc.tile_pool(name="wo", bufs=1) as wopool,
                tc.tile_pool(name="agc", bufs=5) as agpool,
                tc.tile_pool(name="st", bufs=3, space="PSUM") as stpool,
                tc.tile_pool(name="pv", bufs=2, space="PSUM") as pvpool,
                tc.tile_pool(name="rs", bufs=1, space="PSUM") as rspool,
                tc.tile_pool(name="p3", bufs=2, space="PSUM") as pp3,
                tc.tile_pool(name="pt", bufs=6) as ptpool,
                tc.tile_pool(name="ep", bufs=2) as eppool,
                tc.tile_pool(name="o3", bufs=5) as opool,
            ):
                wot = wopool.tile([128, KD, QW], dt.bfloat16)
                nc.sync.dma_start(
                    wot[:], wot_d[:, :].rearrange("(k p) n -> p k n", p=128))

                def epilogue_a(sc, h, pv, rs):
                    # reciprocal as soon as rowsums land (frees rs quickly);
                    # approx_fast (~18 bits) is plenty for softmax denominators
                    rec = eppool.tile([1, SC], dt.float32, tag="rec")
                    nc.vector.reciprocal_approx_fast(rec[:], rs[:])
                    return (sc, h, pv, rec)

                def epilogue_b(sc, h, pv, rec):
                    # normalize columns of attnT by 1/rowsum; the rank-1
                    # broadcast rides PE (cheap) well after rec is ready
                    ssl = slice(sc * SC, (sc + 1) * SC)
                    bc = pp3.tile([128, SC], dt.float32, tag="ps3")
                    nc.tensor.matmul(bc[:], onesr[:], rec[:], start=True, stop=True)
                    bcs = eppool.tile([128, SC], dt.float32, tag="bcs")
                    nc.scalar.copy(bcs[:], bc[:])
                    nc.vector.tensor_mul(att[h][:, ssl], pv[:], bcs[:])

                def allgather_heads(sc, heads):
                    # gather this core's att rows for `heads`; out block r
                    # covers global i-tiles {4r+h for h in heads}
                    ssl = slice(sc * SC, (sc + 1) * SC)
                    nh = len(heads)
                    sfx = f"{sc}h{heads[0]}"
                    ag_in = dpool.tile([nh * HD, SC], dt.bfloat16,
                                       name=f"agi{sfx}", tag=f"agi{sfx}")
                    ag_out = dpool.tile([NCORES * nh * HD, SC], dt.bfloat16,
                                        name=f"ago{sfx}", tag=f"ago{sfx}",
                                        addr_space="Shared")
                    for i, h in enumerate(heads):
                        nc.sync.dma_start(ag_in[i * HD:(i + 1) * HD, :],
                                          att[h][:, ssl])
                    nc.gpsimd.collective_compute(
                        "AllGather",
                        mybir.AluOpType.bypass,
                        replica_groups=[list(range(NCORES))],
                        ins=[ag_in.opt()],
                        outs=[ag_out.opt()],
                    )
                    # preload the gathered chunk into SBUF right away so
                    # out-proj never waits on this DMA
                    nh = len(heads)
                    ag_r = ag_out[:, :].rearrange("(m p) s -> p m s", p=128)
                    agc = agpool.tile([128, NCORES * nh, SC], dt.bfloat16,
                                      tag="agc")
                    nc.sync.dma_start(agc[:], ag_r[:, :, :])
                    return (agc, heads)

                def outproj(sc, pieces):
                    # two half-accumulations bridged through SBUF, so the
                    # a-half matmuls run while the b-half gather is in flight
                    ssl = slice(sc * SC, (sc + 1) * SC)
                    loaded = pieces
                    parts = []
                    agc_a, heads_a = loaded[0]
                    nh_a = len(heads_a)
                    for oc in range(4):
                        ps = pp3.tile([128, SC], dt.float32, tag="ps3")
                        for m in range(NCORES * nh_a):
                            kg = (m // nh_a) * HL + heads_a[m % nh_a]
                            nc.tensor.matmul(
                                ps[:], wot[:, kg, oc * 128:(oc + 1) * 128],
                                agc_a[:, m, :],
                                start=(m == 0), stop=(m == NCORES * nh_a - 1))
                        oa = opool.tile([128, SC], dt.float32, tag="oa")
                        nc.vector.tensor_copy(oa[:], ps[:])
                        parts.append(oa)
                    agc_b, heads_b = loaded[1]
                    nh_b = len(heads_b)
                    for oc in range(4):
                        ps = pp3.tile([128, SC], dt.float32, tag="ps3")
                        for m in range(NCORES * nh_b):
                            kg = (m // nh_b) * HL + heads_b[m % nh_b]
                            nc.tensor.matmul(
                                ps[:], wot[:, kg, oc * 128:(oc + 1) * 128],
                                agc_b[:, m, :],
                                start=(m == 0), stop=(m == NCORES * nh_b - 1))
                        ot = opool.tile([128, SC], dt.float32, tag="ot")
                        nc.vector.tensor_add(ot[:], ps[:], parts[oc][:])
                        nc.sync.dma_start(out_d[oc * 128:(oc + 1) * 128, ssl], ot[:])

                def scores_exp(sc, h, ti):
                    # emits scores matmul + diag mask + exp; returns (pt, v0)
                    d_off = ti * 128 - sc * SC
                    v0 = max(d_off, 0)
                    vsl = slice(v0, SC)
                    qcl = slice(sc * SC + v0, (sc + 1) * SC)
                    st = stpool.tile([128, SC], dt.float32, tag="st")
                    nc.tensor.matmul(st[:, vsl],
                                     kt[:, ti * 128:(ti + 1) * 128],
                                     qt[h][:, qcl], start=True, stop=True)
                    if d_off >= 0:
                        nc.vector.tensor_add(st[:, d_off:d_off + 128],
                                             st[:, d_off:d_off + 128],
                                             dmask[:])
                    pt = ptpool.tile([128, SC], dt.bfloat16, tag="pt")
                    nc.scalar.activation(pt[:, vsl], st[:, vsl],
                                         mybir.ActivationFunctionType.Exp,
                                         scale=SCALE)
                    return pt, v0

                pending_op = None   # (sc, pieces) awaiting outproj
                pending_ep = None   # deferred epilogue_b
                ag_a_cur = None
                LOOKAHEAD = 3
                for sc in range(NSC):
                    n_t = sc * 4 + 4
                    queue = [(h, ti) for h in range(HL) for ti in range(n_t)]
                    cache = {}
                    emitted = 0
                    pvrs = {}
                    for idx, (h, ti) in enumerate(queue):
                        while emitted <= min(idx + LOOKAHEAD, len(queue) - 1):
                            hq, tq = queue[emitted]
                            cache[(hq, tq)] = scores_exp(sc, hq, tq)
                            emitted += 1
                        if ti == 0:
                            pvrs[h] = (pvpool.tile([128, SC], dt.float32,
                                                   name="pv", tag="pv"),
                                       rspool.tile([1, SC], dt.float32,
                                                   name="rs", tag="rs"))
                        pv, rs = pvrs[h]
                        pt, v0 = cache.pop((h, ti))
                        vsl = slice(v0, SC)
                        nc.tensor.matmul(rs[:, vsl], onesc[:], pt[:, vsl],
                                         start=(ti == 0), stop=(ti == n_t - 1))
                        nc.tensor.matmul(pv[:, vsl], vv[:, ti, :], pt[:, vsl],
                                         start=(ti == 0), stop=(ti == n_t - 1))
                        if ti == n_t - 1:
                            ep = epilogue_a(sc, h, pv, rs)
                            if pending_ep is not None:
                                epilogue_b(*pending_ep)
                                pending_ep = None
                            pending_ep = ep
                            if h == 2:
                                # epilogue_b(h0) and (h1) are done by now
                                ag_a_cur = allgather_heads(sc, [0, 1])
                            if h == 3 and pending_op is not None:
                                outproj(*pending_op)
                                pending_op = None
                    epilogue_b(*pending_ep)
                    pending_ep = None
                    ag_b_cur = allgather_heads(sc, [2, 3])
                    pending_op = (sc, [ag_a_cur, ag_b_cur])
                outproj(*pending_op)
    if not nc.is_finalized():
        nc.finalize()
    return nc


_CACHE = {}


def _get_nc():
    if "nc" not in _CACHE:
        _CACHE["nc"] = _build_nc()
    return _CACHE["nc"]


def _prep_in_maps(x, wq, wk, wv, wo, freqs_cos, freqs_sin):
    xt = np.ascontiguousarray(x.reshape(S, D).T).astype(BF)
    cosd = np.repeat(np.asarray(freqs_cos, np.float32).T, 2, axis=0).astype(BF)
    sind = np.repeat(np.asarray(freqs_sin, np.float32).T, 2, axis=0).astype(BF)
    swapt = np.zeros((HD, HD), np.float32)
    for i in range(HD // 2):
        swapt[2 * i + 1, 2 * i] = -1.0
        swapt[2 * i, 2 * i + 1] = 1.0
    swapt = swapt.astype(BF)
    ident = np.eye(HD, dtype=np.float32).astype(BF)
    t_idx = np.arange(128)[:, None]
    s_idx = np.arange(128)[None, :]
    dmask = np.where(s_idx >= t_idx, 0.0, NEG).astype(np.float32)
    onesc = np.ones((128, 1), np.float32).astype(BF)
    onesr = np.ones((1, 128), np.float32)

    wq = np.asarray(wq, np.float32)
    wk = np.asarray(wk, np.float32)
    wv = np.asarray(wv, np.float32)
    wo = np.asarray(wo, np.float32)

    in_maps = []
    for c in range(NCORES):
        qsl = slice(QW * c, QW * (c + 1))
        ksl = slice(HD * c, HD * (c + 1))
        in_maps.append({
            "xt": xt,
            "wqt": np.ascontiguousarray(wq[qsl].T).astype(BF),
            "wkt": np.ascontiguousarray(wk[ksl].T).astype(BF),
            "wvt": np.ascontiguousarray(wv[ksl].T).astype(BF),
            "wot": np.ascontiguousarray(wo[qsl].T).astype(BF),
            "cosd": cosd, "sind": sind, "swapt": swapt, "ident": ident,
            "dmask": dmask, "onesc": onesc, "onesr": onesr,
        })
    return in_maps


def _patch_ldw_opt():
    # walrus's LDWEIGHTS background-buffer optimization is disabled by
    # default in this flow; without it every matmul serializes a ~103ns
    # weight load. Flip the flag; the accuracy gate guards correctness.
    from concourse import bass_utils
    if getattr(bass_utils, "_ldw_patched", False):
        return
    orig = bass_utils.run_command

    def run_command_ldw(cmd, *a, **k):
        # (ldw-opt=true miscompiles: walrus visitInstLdweights error)
        return orig(cmd, *a, **k)

    bass_utils.run_command = run_command_ldw
    bass_utils._ldw_patched = True


def run(inputs, trace=False):
    _patch_ldw_opt()
    from concourse.bass_utils import run_bass_kernel_spmd
    nc = _get_nc()
    in_maps = _prep_in_maps(
        inputs["x"], inputs["wq"], inputs["wk"], inputs["wv"], inputs["wo"],
        inputs["freqs_cos"], inputs["freqs_sin"])
    res = run_bass_kernel_spmd(nc, in_maps, core_ids=list(range(NCORES)),
                               trace=trace)
    shards = [np.asarray(res.results[c]["out_t"], np.float32)
              for c in range(NCORES)]
    full = np.concatenate(shards, axis=0)          # [4096, 2048]
    out = np.ascontiguousarray(full.T)[None]       # [1, 2048, 4096]
    return out.astype(np.float32), res


def kernel(**inputs):
    out, _ = run(inputs, trace=False)
    return out


# revision 44
# speedup vs baseline: 1.0094x; 1.0094x over previous
"""Distributed GQA attention kernel for 8 TRN2 NeuronCores.

Problem: B=1, S=2048, D=4096, H=32 q-heads, KV=8 kv-heads, HD=128.
  q = rope(x@wq.T), k = rope(x@wk.T), v = x@wv.T
  out = softmax(causal(q@k.T/sqrt(HD))) @ v @ wo.T

Sharding: tensor-parallel over heads. Core c owns q-heads 4c..4c+3 and
kv-head c. Device-side per core:
  phase 1: QT/KT (rope'd, [hd, s] layout) + VT projections; V tiles
           ([t, hd]) via DMA transpose
  phase 2: causal attention producing attnT chunks; software-pipelined:
           epilogues deferred one head, AllGather + out-proj deferred
           one s-chunk so they overlap the next chunk's attention.
Host side: layout prep (transposes, bf16 cast, rope tables) + final
concat/transpose of the 8 out.T slices.
"""

import math
import numpy as np
import ml_dtypes

BF = ml_dtypes.bfloat16

B, S, D = 1, 2048, 4096
H, KV, HD = 32, 8, 128
NCORES = 8
HL = H // NCORES            # 4 local q heads
QW = HL * HD                # 512 local q width
SC = 512                    # s-chunk width
NSC = S // SC               # 4 s-chunks
KD = 32                     # d-dim k-tiles (4096/128)
NT = S // 128               # 16 t-tiles
SCALE = 1.0 / math.sqrt(HD)
NEG = -30000.0


def _build_nc():
    import concourse.bass as bass
    import concourse.mybir as mybir
    from concourse import bacc, tile

    dt = mybir.dt
    nc = bacc.Bacc()

    xt_d = nc.declare_dram_parameter("xt", [D, S], dt.bfloat16, isOutput=False)
    wqt_d = nc.declare_dram_parameter("wqt", [D, QW], dt.bfloat16, isOutput=False)
    wkt_d = nc.declare_dram_parameter("wkt", [D, HD], dt.bfloat16, isOutput=False)
    wvt_d = nc.declare_dram_parameter("wvt", [D, HD], dt.bfloat16, isOutput=False)
    wot_d = nc.declare_dram_parameter("wot", [D, QW], dt.bfloat16, isOutput=False)
    cosd_d = nc.declare_dram_parameter("cosd", [HD, S], dt.bfloat16, isOutput=False)
    sind_d = nc.declare_dram_parameter("sind", [HD, S], dt.bfloat16, isOutput=False)
    swapt_d = nc.declare_dram_parameter("swapt", [HD, HD], dt.bfloat16, isOutput=False)
    ident_d = nc.declare_dram_parameter("ident", [HD, HD], dt.bfloat16, isOutput=False)
    dmask_d = nc.declare_dram_parameter("dmask", [128, 128], dt.float32, isOutput=False)
    onesc_d = nc.declare_dram_parameter("onesc", [128, 1], dt.bfloat16, isOutput=False)
    onesr_d = nc.declare_dram_parameter("onesr", [1, 128], dt.float32, isOutput=False)
    out_d = nc.declare_dram_parameter("out_t", [QW, S], dt.float32, isOutput=True)

    with tile.TileContext(nc) as tc:
        with (
            tc.tile_pool(name="const", bufs=1) as cpool,
            tc.tile_pool(name="qkv", bufs=1) as qkvpool,
            tc.tile_pool(name="att", bufs=1) as attpool,
            tc.tile_pool(name="dram", bufs=1, space="DRAM") as dpool,
        ):
            # ---- small resident constants ----
            cosd = cpool.tile([HD, S], dt.bfloat16)
            sind = cpool.tile([HD, S], dt.bfloat16)
            swapt = cpool.tile([HD, HD], dt.bfloat16)
            ident = cpool.tile([HD, HD], dt.bfloat16)
            dmask = cpool.tile([128, 128], dt.float32)
            onesc = cpool.tile([128, 1], dt.bfloat16)
            onesr = cpool.tile([1, 128], dt.float32)
            nc.sync.dma_start(cosd[:], cosd_d[:, :])
            nc.sync.dma_start(sind[:], sind_d[:, :])
            nc.sync.dma_start(swapt[:], swapt_d[:, :])
            nc.sync.dma_start(ident[:], ident_d[:, :])
            nc.sync.dma_start(dmask[:], dmask_d[:, :])
            nc.sync.dma_start(onesc[:], onesc_d[:, :])
            nc.sync.dma_start(onesr[:], onesr_d[:, :])
            # warm up the ACT exp table load before attention needs it
            warm = cpool.tile([1, 1], dt.float32)
            nc.scalar.activation(warm[:], onesr[0:1, 0:1],
                                 mybir.ActivationFunctionType.Exp)

            # ---- persistent activations ----
            qt = [qkvpool.tile([HD, S], dt.bfloat16, name=f"qt{h}", tag=f"qt{h}")
                  for h in range(HL)]
            kt = qkvpool.tile([HD, S], dt.bfloat16)
            vv = qkvpool.tile([128, NT, HD], dt.bfloat16)   # [t_part, ti, hd]
            att = [attpool.tile([HD, S], dt.bfloat16, name=f"att{h}", tag=f"att{h}")
                   for h in range(HL)]

            xt_r = xt_d[:, :].rearrange("(k p) s -> p k s", p=128)

            # ================= phase 1: projections + rope =================
            with (
                tc.tile_pool(name="w1", bufs=1) as wpool,
                tc.tile_pool(name="xc", bufs=2) as xpool,
                tc.tile_pool(name="p1", bufs=3, space="PSUM") as pp1,
                tc.tile_pool(name="pr", bufs=2, space="PSUM") as ppr,
                tc.tile_pool(name="rtmp", bufs=3) as rtpool,
            ):
                wqt = wpool.tile([128, KD, QW], dt.bfloat16)
                wkt = wpool.tile([128, KD, HD], dt.bfloat16)
                wvt = wpool.tile([128, KD, HD], dt.bfloat16)
                vt = wpool.tile([HD, S], dt.bfloat16)
                wqt_r = wqt_d[:, :].rearrange("(k p) n -> p k n", p=128)
                # interleave first x-chunk with weight loads so the first
                # matmuls can start as early as possible
                xc0 = xpool.tile([128, KD, SC], dt.bfloat16, tag="xc")
                pieces = [(0, 2), (2, 4), (4, 8), (8, 12), (12, 16),
                          (16, 20), (20, 24), (24, 28), (28, 32)]
                for lo, hi in pieces:
                    ksl = slice(lo, hi)
                    nc.sync.dma_start(xc0[:, ksl, :], xt_r[:, ksl, 0:SC])
                    nc.sync.dma_start(wqt[:, ksl, :], wqt_r[:, ksl, :])
                nc.sync.dma_start(
                    wkt[:], wkt_d[:, :].rearrange("(k p) n -> p k n", p=128))
                nc.sync.dma_start(
                    wvt[:], wvt_d[:, :].rearrange("(k p) n -> p k n", p=128))

                for sc in range(NSC):
                    ssl = slice(sc * SC, (sc + 1) * SC)
                    if sc == 0:
                        xc = xc0
                    else:
                        xc = xpool.tile([128, KD, SC], dt.bfloat16, tag="xc")
                        for kg in range(4):
                            ksl = slice(kg * 8, (kg + 1) * 8)
                            nc.sync.dma_start(xc[:, ksl, :], xt_r[:, ksl, ssl])

                    # 4 Q heads (rope), K (rope), V (plain) — all [hd, s]
                    for hi in range(HL + 2):
                        ps = pp1.tile([128, SC], dt.float32)
                        for k in range(KD):
                            if hi < HL:
                                lhs = wqt[:, k, hi * HD:(hi + 1) * HD]
                            elif hi == HL:
                                lhs = wkt[:, k, :]
                            else:
                                lhs = wvt[:, k, :]
                            nc.tensor.matmul(ps[:], lhs, xc[:, k, :],
                                             start=(k == 0), stop=(k == KD - 1))
                        if hi == HL + 1:
                            nc.scalar.copy(vt[:, ssl], ps[:])
                            continue
                        # rope: out = q*cos + rot(q)*sin, rot via swap-matmul
                        qs = rtpool.tile([128, SC], dt.bfloat16, tag="ropeqs")
                        qc = rtpool.tile([128, SC], dt.bfloat16, tag="ropeqc")
                        nc.vector.tensor_mul(qs[:], ps[:], sind[:, ssl])
                        nc.vector.tensor_mul(qc[:], ps[:], cosd[:, ssl])
                        ps2 = ppr.tile([128, SC], dt.float32)
                        nc.tensor.matmul(ps2[:], swapt[:], qs[:], start=True, stop=False)
                        nc.tensor.matmul(ps2[:], ident[:], qc[:], start=False, stop=True)
                        dst = qt[hi] if hi < HL else kt
                        nc.scalar.copy(dst[:, ssl], ps2[:])

                    # V tiles in [t, hd] layout via DMA transpose
                    for vtile in range(4):
                        ti = sc * 4 + vtile
                        nc.sync.dma_start_transpose(
                            vv[:, ti, :], vt[:, ti * 128:(ti + 1) * 128])

            # ============ phase 2+3: attention, allgather, out-proj ============
            with (
                tc.tile_pool(name="wo", bufs=1) as wopool,
                tc.tile_pool(name="agc", bufs=5) as agpool,
                tc.tile_pool(name="st", bufs=3, space="PSUM") as stpool,
                tc.tile_pool(name="pv", bufs=2, space="PSUM") as pvpool,
                tc.tile_pool(name="rs", bufs=1, space="PSUM") as rspool,
                tc.tile_pool(name="p3", bufs=2, space="PSUM") as pp3,
                tc.tile_pool(name="pt", bufs=6) as ptpool,
                tc.tile_pool(name="ep", bufs=2) as eppool,
                tc.tile_pool(name="o3", bufs=5) as opool,
            ):
                wot = wopool.tile([128, KD, QW], dt.bfloat16)
                nc.sync.dma_start(
                    wot[:], wot_d[:, :].rearrange("(k p) n -> p k n", p=128))

                def epilogue_a(sc, h, pv, rs):
                    # reciprocal as soon as rowsums land (frees rs quickly);
                    # approx_fast (~18 bits) is plenty for softmax denominators
                    rec = eppool.tile([1, SC], dt.float32, tag="rec")
                    nc.vector.reciprocal_approx_fast(rec[:], rs[:])
                    return (sc, h, pv, rec)

                def epilogue_b(sc, h, pv, rec):
                    # normalize columns of attnT by 1/rowsum; the rank-1
                    # broadcast rides PE (cheap) well after rec is ready
                    ssl = slice(sc * SC, (sc + 1) * SC)
                    bc = pp3.tile([128, SC], dt.float32, tag="ps3")
                    nc.tensor.matmul(bc[:], onesr[:], rec[:], start=True, stop=True)
                    bcs = eppool.tile([128, SC], dt.float32, tag="bcs")
                    nc.scalar.copy(bcs[:], bc[:])
                    nc.vector.tensor_mul(att[h][:, ssl], pv[:], bcs[:])

                def allgather_heads(sc, heads):
                    # gather this core's att rows for `heads`; out block r
                    # covers global i-tiles {4r+h for h in heads}
                    ssl = slice(sc * SC, (sc + 1) * SC)
                    nh = len(heads)
                    sfx = f"{sc}h{heads[0]}"
                    ag_in = dpool.tile([nh * HD, SC], dt.bfloat16,
                                       name=f"agi{sfx}", tag=f"agi{sfx}")
                    ag_out = dpool.tile([NCORES * nh * HD, SC], dt.bfloat16,
                                        name=f"ago{sfx}", tag=f"ago{sfx}",
                                        addr_space="Shared")
                    for i, h in enumerate(heads):
                        nc.sync.dma_start(ag_in[i * HD:(i + 1) * HD, :],
                                          att[h][:, ssl])
                    nc.gpsimd.collective_compute(
                        "AllGather",
                        mybir.AluOpType.bypass,
                        replica_groups=[list(range(NCORES))],
                        ins=[ag_in.opt()],
                        outs=[ag_out.opt()],
                    )
                    # preload the gathered chunk into SBUF right away so
                    # out-proj never waits on this DMA
                    nh = len(heads)
                    ag_r = ag_out[:, :].rearrange("(m p) s -> p m s", p=128)
                    agc = agpool.tile([128, NCORES * nh, SC], dt.bfloat16,
                                      tag="agc")
                    nc.sync.dma_start(agc[:], ag_r[:, :, :])
                    return (agc, heads)

                def outproj(sc, pieces):
                    # two half-accumulations bridged through SBUF, so the
                    # a-half matmuls run while the b-half gather is in flight
                    ssl = slice(sc * SC, (sc + 1) * SC)
                    loaded = pieces
                    parts = []
                    agc_a, heads_a = loaded[0]
                    nh_a = len(heads_a)
                    for oc in range(4):
                        ps = pp3.tile([128, SC], dt.float32, tag="ps3")
                        for m in range(NCORES * nh_a):
                            kg = (m // nh_a) * HL + heads_a[m % nh_a]
                            nc.tensor.matmul(
                                ps[:], wot[:, kg, oc * 128:(oc + 1) * 128],
                                agc_a[:, m, :],
                                start=(m == 0), stop=(m == NCORES * nh_a - 1))
                        oa = opool.tile([128, SC], dt.float32, tag="oa")
                        nc.vector.tensor_copy(oa[:], ps[:])
                        parts.append(oa)
                    agc_b, heads_b = loaded[1]
                    nh_b = len(heads_b)
                    for oc in range(4):
                        ps = pp3.tile([128, SC], dt.float32, tag="ps3")
                        for m in range(NCORES * nh_b):
                            kg = (m // nh_b) * HL + heads_b[m % nh_b]
                            nc.tensor.matmul(
                                ps[:], wot[:, kg, oc * 128:(oc + 1) * 128],
                                agc_b[:, m, :],
                                start=(m == 0), stop=(m == NCORES * nh_b - 1))
                        ot = opool.tile([128, SC], dt.float32, tag="ot")
                        nc.vector.tensor_add(ot[:], ps[:], parts[oc][:])
                        nc.sync.dma_start(out_d[oc * 128:(oc + 1) * 128, ssl], ot[:])

                def scores_exp(sc, h, ti):
                    # emits scores matmul + diag mask + exp; returns (pt, v0)
                    d_off = ti * 128 - sc * SC
                    v0 = max(d_off, 0)
                    vsl = slice(v0, SC)
                    qcl = slice(sc * SC + v0, (sc + 1) * SC)
                    st = stpool.tile([128, SC], dt.float32, tag="st")
                    nc.tensor.matmul(st[:, vsl],
                                     kt[:, ti * 128:(ti + 1) * 128],
                                     qt[h][:, qcl], start=True, stop=True)
                    if d_off >= 0:
                        nc.vector.tensor_add(st[:, d_off:d_off + 128],
                                             st[:, d_off:d_off + 128],
                                             dmask[:])
                    pt = ptpool.tile([128, SC], dt.bfloat16, tag="pt")
                    nc.scalar.activation(pt[:, vsl], st[:, vsl],
                                         mybir.ActivationFunctionType.Exp,
                                         scale=SCALE)
                    return pt, v0

                pending_ops = {}    # sc -> (sc, pieces) awaiting outproj
                # emission slots (sc, h) -> which chunk's outproj to emit;
                # placed so each gather has ample time to complete first
                emit_map = {(2, 0): 0, (2, 3): 1, (3, 3): 2}
                pending_ep = None   # deferred epilogue_b
                ag_a_cur = None
                LOOKAHEAD = 3
                for sc in range(NSC):
                    n_t = sc * 4 + 4
                    queue = [(h, ti) for h in range(HL) for ti in range(n_t)]
                    cache = {}
                    emitted = 0
                    pvrs = {}
                    for idx, (h, ti) in enumerate(queue):
                        while emitted <= min(idx + LOOKAHEAD, len(queue) - 1):
                            hq, tq = queue[emitted]
                            cache[(hq, tq)] = scores_exp(sc, hq, tq)
                            emitted += 1
                        if ti == 0:
                            pvrs[h] = (pvpool.tile([128, SC], dt.float32,
                                                   name="pv", tag="pv"),
                                       rspool.tile([1, SC], dt.float32,
                                                   name="rs", tag="rs"))
                        pv, rs = pvrs[h]
                        pt, v0 = cache.pop((h, ti))
                        vsl = slice(v0, SC)
                        nc.tensor.matmul(rs[:, vsl], onesc[:], pt[:, vsl],
                                         start=(ti == 0), stop=(ti == n_t - 1))
                        nc.tensor.matmul(pv[:, vsl], vv[:, ti, :], pt[:, vsl],
                                         start=(ti == 0), stop=(ti == n_t - 1))
                        if ti == n_t - 1:
                            ep = epilogue_a(sc, h, pv, rs)
                            if pending_ep is not None:
                                epilogue_b(*pending_ep)
                                pending_ep = None
                            pending_ep = ep
                            if h == 2:
                                # epilogue_b(h0) and (h1) are done by now
                                ag_a_cur = allgather_heads(sc, [0, 1])
                            opsc = emit_map.get((sc, h))
                            if opsc is not None and opsc in pending_ops:
                                outproj(*pending_ops.pop(opsc))
                    epilogue_b(*pending_ep)
                    pending_ep = None
                    ag_b_cur = allgather_heads(sc, [2, 3])
                    pending_ops[sc] = (sc, [ag_a_cur, ag_b_cur])
                for opsc in sorted(pending_ops):
                    outproj(*pending_ops[opsc])
    if not nc.is_finalized():
        nc.finalize()
    return nc


_CACHE = {}


def _get_nc():
    if "nc" not in _CACHE:
        _CACHE["nc"] = _build_nc()
    return _CACHE["nc"]


def _prep_in_maps(x, wq, wk, wv, wo, freqs_cos, freqs_sin):
    xt = np.ascontiguousarray(x.reshape(S, D).T).astype(BF)
    cosd = np.repeat(np.asarray(freqs_cos, np.float32).T, 2, axis=0).astype(BF)
    sind = np.repeat(np.asarray(freqs_sin, np.float32).T, 2, axis=0).astype(BF)
    swapt = np.zeros((HD, HD), np.float32)
    for i in range(HD // 2):
        swapt[2 * i + 1, 2 * i] = -1.0
        swapt[2 * i, 2 * i + 1] = 1.0
    swapt = swapt.astype(BF)
    ident = np.eye(HD, dtype=np.float32).astype(BF)
    t_idx = np.arange(128)[:, None]
    s_idx = np.arange(128)[None, :]
    dmask = np.where(s_idx >= t_idx, 0.0, NEG).astype(np.float32)
    onesc = np.ones((128, 1), np.float32).astype(BF)
    onesr = np.ones((1, 128), np.float32)

    wq = np.asarray(wq, np.float32)
    wk = np.asarray(wk, np.float32)
    wv = np.asarray(wv, np.float32)
    wo = np.asarray(wo, np.float32)

    in_maps = []
    for c in range(NCORES):
        qsl = slice(QW * c, QW * (c + 1))
        ksl = slice(HD * c, HD * (c + 1))
        in_maps.append({
            "xt": xt,
            "wqt": np.ascontiguousarray(wq[qsl].T).astype(BF),
            "wkt": np.ascontiguousarray(wk[ksl].T).astype(BF),
            "wvt": np.ascontiguousarray(wv[ksl].T).astype(BF),
            "wot": np.ascontiguousarray(wo[qsl].T).astype(BF),
            "cosd": cosd, "sind": sind, "swapt": swapt, "ident": ident,
            "dmask": dmask, "onesc": onesc, "onesr": onesr,
        })
    return in_maps


def _patch_ldw_opt():
    # walrus's LDWEIGHTS background-buffer optimization is disabled by
    # default in this flow; without it every matmul serializes a ~103ns
    # weight load. Flip the flag; the accuracy gate guards correctness.
    from concourse import bass_utils
    if getattr(bass_utils, "_ldw_patched", False):
        return
    orig = bass_utils.run_command

    def run_command_ldw(cmd, *a, **k):
        # (ldw-opt=true miscompiles: walrus visitInstLdweights error)
        return orig(cmd, *a, **k)

    bass_utils.run_command = run_command_ldw
    bass_utils._ldw_patched = True


def run(inputs, trace=False):
    _patch_ldw_opt()
    from concourse.bass_utils import run_bass_kernel_spmd
    nc = _get_nc()
    in_maps = _prep_in_maps(
        inputs["x"], inputs["wq"], inputs["wk"], inputs["wv"], inputs["wo"],
        inputs["freqs_cos"], inputs["freqs_sin"])
    res = run_bass_kernel_spmd(nc, in_maps, core_ids=list(range(NCORES)),
                               trace=trace)
    shards = [np.asarray(res.results[c]["out_t"], np.float32)
              for c in range(NCORES)]
    full = np.concatenate(shards, axis=0)          # [4096, 2048]
    out = np.ascontiguousarray(full.T)[None]       # [1, 2048, 4096]
    return out.astype(np.float32), res


def kernel(**inputs):
    out, _ = run(inputs, trace=False)
    return out


# revision 45
# speedup vs baseline: 1.0281x; 1.0186x over previous
"""Distributed GQA attention kernel for 8 TRN2 NeuronCores.

Problem: B=1, S=2048, D=4096, H=32 q-heads, KV=8 kv-heads, HD=128.
  q = rope(x@wq.T), k = rope(x@wk.T), v = x@wv.T
  out = softmax(causal(q@k.T/sqrt(HD))) @ v @ wo.T

Sharding: tensor-parallel over heads. Core c owns q-heads 4c..4c+3 and
kv-head c. Device-side per core:
  phase 1: QT/KT (rope'd, [hd, s] layout) + VT projections; V tiles
           ([t, hd]) via DMA transpose
  phase 2: causal attention producing attnT chunks; software-pipelined:
           epilogues deferred one head, AllGather + out-proj deferred
           one s-chunk so they overlap the next chunk's attention.
Host side: layout prep (transposes, bf16 cast, rope tables) + final
concat/transpose of the 8 out.T slices.
"""

import math
import numpy as np
import ml_dtypes

BF = ml_dtypes.bfloat16

B, S, D = 1, 2048, 4096
H, KV, HD = 32, 8, 128
NCORES = 8
HL = H // NCORES            # 4 local q heads
QW = HL * HD                # 512 local q width
SC = 512                    # s-chunk width
NSC = S // SC               # 4 s-chunks
KD = 32                     # d-dim k-tiles (4096/128)
NT = S // 128               # 16 t-tiles
SCALE = 1.0 / math.sqrt(HD)
NEG = -30000.0


def _build_nc():
    import concourse.bass as bass
    import concourse.mybir as mybir
    from concourse import bacc, tile

    dt = mybir.dt
    nc = bacc.Bacc()

    xt_d = nc.declare_dram_parameter("xt", [D, S], dt.bfloat16, isOutput=False)
    wqt_d = nc.declare_dram_parameter("wqt", [D, QW], dt.bfloat16, isOutput=False)
    wkt_d = nc.declare_dram_parameter("wkt", [D, HD], dt.bfloat16, isOutput=False)
    wvt_d = nc.declare_dram_parameter("wvt", [D, HD], dt.bfloat16, isOutput=False)
    wot_d = nc.declare_dram_parameter("wot", [D, QW], dt.bfloat16, isOutput=False)
    cosd_d = nc.declare_dram_parameter("cosd", [HD, S], dt.bfloat16, isOutput=False)
    sind_d = nc.declare_dram_parameter("sind", [HD, S], dt.bfloat16, isOutput=False)
    swapt_d = nc.declare_dram_parameter("swapt", [HD, HD], dt.bfloat16, isOutput=False)
    ident_d = nc.declare_dram_parameter("ident", [HD, HD], dt.bfloat16, isOutput=False)
    dmask_d = nc.declare_dram_parameter("dmask", [128, 128], dt.float32, isOutput=False)
    onesc_d = nc.declare_dram_parameter("onesc", [128, 1], dt.bfloat16, isOutput=False)
    onesr_d = nc.declare_dram_parameter("onesr", [1, 128], dt.float32, isOutput=False)
    out_d = nc.declare_dram_parameter("out_t", [QW, S], dt.float32, isOutput=True)

    with tile.TileContext(nc) as tc:
        with (
            tc.tile_pool(name="const", bufs=1) as cpool,
            tc.tile_pool(name="qkv", bufs=1) as qkvpool,
            tc.tile_pool(name="att", bufs=1) as attpool,
            tc.tile_pool(name="dram", bufs=1, space="DRAM") as dpool,
        ):
            # ---- small resident constants ----
            cosd = cpool.tile([HD, S], dt.bfloat16)
            sind = cpool.tile([HD, S], dt.bfloat16)
            swapt = cpool.tile([HD, HD], dt.bfloat16)
            ident = cpool.tile([HD, HD], dt.bfloat16)
            dmask = cpool.tile([128, 128], dt.float32)
            onesc = cpool.tile([128, 1], dt.bfloat16)
            onesr = cpool.tile([1, 128], dt.float32)
            nc.sync.dma_start(cosd[:], cosd_d[:, :])
            nc.sync.dma_start(sind[:], sind_d[:, :])
            nc.sync.dma_start(swapt[:], swapt_d[:, :])
            nc.sync.dma_start(ident[:], ident_d[:, :])
            nc.sync.dma_start(dmask[:], dmask_d[:, :])
            nc.sync.dma_start(onesc[:], onesc_d[:, :])
            nc.sync.dma_start(onesr[:], onesr_d[:, :])
            # warm up the ACT exp table load before attention needs it
            warm = cpool.tile([1, 1], dt.float32)
            nc.scalar.activation(warm[:], onesr[0:1, 0:1],
                                 mybir.ActivationFunctionType.Exp)

            # ---- persistent activations ----
            qt = [qkvpool.tile([HD, S], dt.bfloat16, name=f"qt{h}", tag=f"qt{h}")
                  for h in range(HL)]
            kt = qkvpool.tile([HD, S], dt.bfloat16)
            vv = qkvpool.tile([128, NT, HD], dt.bfloat16)   # [t_part, ti, hd]
            att = [attpool.tile([HD, S], dt.bfloat16, name=f"att{h}", tag=f"att{h}")
                   for h in range(HL)]

            xt_r = xt_d[:, :].rearrange("(k p) s -> p k s", p=128)

            # ================= phase 1: projections + rope =================
            with (
                tc.tile_pool(name="w1", bufs=1) as wpool,
                tc.tile_pool(name="xc", bufs=2) as xpool,
                tc.tile_pool(name="p1", bufs=3, space="PSUM") as pp1,
                tc.tile_pool(name="pr", bufs=2, space="PSUM") as ppr,
                tc.tile_pool(name="rtmp", bufs=3) as rtpool,
            ):
                wqt = wpool.tile([128, KD, QW], dt.bfloat16)
                wkt = wpool.tile([128, KD, HD], dt.bfloat16)
                wvt = wpool.tile([128, KD, HD], dt.bfloat16)
                vt = wpool.tile([HD, S], dt.bfloat16)
                wqt_r = wqt_d[:, :].rearrange("(k p) n -> p k n", p=128)
                # interleave first x-chunk with weight loads so the first
                # matmuls can start as early as possible
                xc0 = xpool.tile([128, KD, SC], dt.bfloat16, tag="xc")
                pieces = [(0, 2), (2, 4), (4, 8), (8, 12), (12, 16),
                          (16, 20), (20, 24), (24, 28), (28, 32)]
                for lo, hi in pieces:
                    ksl = slice(lo, hi)
                    nc.sync.dma_start(xc0[:, ksl, :], xt_r[:, ksl, 0:SC])
                    nc.sync.dma_start(wqt[:, ksl, :], wqt_r[:, ksl, :])
                nc.sync.dma_start(
                    wkt[:], wkt_d[:, :].rearrange("(k p) n -> p k n", p=128))
                nc.sync.dma_start(
                    wvt[:], wvt_d[:, :].rearrange("(k p) n -> p k n", p=128))

                for sc in range(NSC):
                    ssl = slice(sc * SC, (sc + 1) * SC)
                    if sc == 0:
                        xc = xc0
                    else:
                        xc = xpool.tile([128, KD, SC], dt.bfloat16, tag="xc")
                        for kg in range(4):
                            ksl = slice(kg * 8, (kg + 1) * 8)
                            nc.sync.dma_start(xc[:, ksl, :], xt_r[:, ksl, ssl])

                    # 4 Q heads (rope), K (rope), V (plain) — all [hd, s]
                    for hi in range(HL + 2):
                        ps = pp1.tile([128, SC], dt.float32)
                        for k in range(KD):
                            if hi < HL:
                                lhs = wqt[:, k, hi * HD:(hi + 1) * HD]
                            elif hi == HL:
                                lhs = wkt[:, k, :]
                            else:
                                lhs = wvt[:, k, :]
                            nc.tensor.matmul(ps[:], lhs, xc[:, k, :],
                                             start=(k == 0), stop=(k == KD - 1))
                        if hi == HL + 1:
                            nc.scalar.copy(vt[:, ssl], ps[:])
                            continue
                        # rope: out = q*cos + rot(q)*sin, rot via swap-matmul
                        qs = rtpool.tile([128, SC], dt.bfloat16, tag="ropeqs")
                        qc = rtpool.tile([128, SC], dt.bfloat16, tag="ropeqc")
                        nc.vector.tensor_mul(qs[:], ps[:], sind[:, ssl])
                        nc.vector.tensor_mul(qc[:], ps[:], cosd[:, ssl])
                        ps2 = ppr.tile([128, SC], dt.float32)
                        nc.tensor.matmul(ps2[:], swapt[:], qs[:], start=True, stop=False)
                        nc.tensor.matmul(ps2[:], ident[:], qc[:], start=False, stop=True)
                        dst = qt[hi] if hi < HL else kt
                        nc.scalar.copy(dst[:, ssl], ps2[:])

                    # V tiles in [t, hd] layout via DMA transpose
                    for vtile in range(4):
                        ti = sc * 4 + vtile
                        nc.sync.dma_start_transpose(
                            vv[:, ti, :], vt[:, ti * 128:(ti + 1) * 128])

            # ============ phase 2+3: attention, allgather, out-proj ============
            with (
                tc.tile_pool(name="wo", bufs=1) as wopool,
                tc.tile_pool(name="agc", bufs=5) as agpool,
                tc.tile_pool(name="st", bufs=3, space="PSUM") as stpool,
                tc.tile_pool(name="pv", bufs=2, space="PSUM") as pvpool,
                tc.tile_pool(name="rs", bufs=1, space="PSUM") as rspool,
                tc.tile_pool(name="p3", bufs=2, space="PSUM") as pp3,
                tc.tile_pool(name="pt", bufs=6) as ptpool,
                tc.tile_pool(name="ep", bufs=2) as eppool,
                tc.tile_pool(name="o3", bufs=5) as opool,
            ):
                wot = wopool.tile([128, KD, QW], dt.bfloat16)
                nc.sync.dma_start(
                    wot[:], wot_d[:, :].rearrange("(k p) n -> p k n", p=128))

                # zero-dependency dummy gather (uninitialized data, result
                # unused) to absorb first-collective setup during phase 1
                cw_in = dpool.tile([128, 16], dt.bfloat16)
                cw_out = dpool.tile([NCORES * 128, 16], dt.bfloat16,
                                    addr_space="Shared")
                nc.gpsimd.collective_compute(
                    "AllGather",
                    mybir.AluOpType.bypass,
                    replica_groups=[list(range(NCORES))],
                    ins=[cw_in.opt()],
                    outs=[cw_out.opt()],
                )

                def epilogue_a(sc, h, pv, rs):
                    # reciprocal as soon as rowsums land (frees rs quickly);
                    # approx_fast (~18 bits) is plenty for softmax denominators
                    rec = eppool.tile([1, SC], dt.float32, tag="rec")
                    nc.vector.reciprocal_approx_fast(rec[:], rs[:])
                    return (sc, h, pv, rec)

                def epilogue_b(sc, h, pv, rec):
                    # normalize columns of attnT by 1/rowsum; the rank-1
                    # broadcast rides PE (cheap) well after rec is ready
                    ssl = slice(sc * SC, (sc + 1) * SC)
                    bc = pp3.tile([128, SC], dt.float32, tag="ps3")
                    nc.tensor.matmul(bc[:], onesr[:], rec[:], start=True, stop=True)
                    bcs = eppool.tile([128, SC], dt.float32, tag="bcs")
                    nc.scalar.copy(bcs[:], bc[:])
                    nc.vector.tensor_mul(att[h][:, ssl], pv[:], bcs[:])

                def allgather_heads(sc, heads):
                    # gather this core's att rows for `heads`; out block r
                    # covers global i-tiles {4r+h for h in heads}
                    ssl = slice(sc * SC, (sc + 1) * SC)
                    nh = len(heads)
                    sfx = f"{sc}h{heads[0]}"
                    ag_in = dpool.tile([nh * HD, SC], dt.bfloat16,
                                       name=f"agi{sfx}", tag=f"agi{sfx}")
                    ag_out = dpool.tile([NCORES * nh * HD, SC], dt.bfloat16,
                                        name=f"ago{sfx}", tag=f"ago{sfx}",
                                        addr_space="Shared")
                    for i, h in enumerate(heads):
                        nc.sync.dma_start(ag_in[i * HD:(i + 1) * HD, :],
                                          att[h][:, ssl])
                    nc.gpsimd.collective_compute(
                        "AllGather",
                        mybir.AluOpType.bypass,
                        replica_groups=[list(range(NCORES))],
                        ins=[ag_in.opt()],
                        outs=[ag_out.opt()],
                    )
                    # preload the gathered chunk into SBUF right away so
                    # out-proj never waits on this DMA
                    nh = len(heads)
                    ag_r = ag_out[:, :].rearrange("(m p) s -> p m s", p=128)
                    agc = agpool.tile([128, NCORES * nh, SC], dt.bfloat16,
                                      tag="agc")
                    nc.sync.dma_start(agc[:], ag_r[:, :, :])
                    return (agc, heads)

                def outproj(sc, pieces):
                    # two half-accumulations bridged through SBUF, so the
                    # a-half matmuls run while the b-half gather is in flight
                    ssl = slice(sc * SC, (sc + 1) * SC)
                    loaded = pieces
                    parts = []
                    agc_a, heads_a = loaded[0]
                    nh_a = len(heads_a)
                    for oc in range(4):
                        ps = pp3.tile([128, SC], dt.float32, tag="ps3")
                        for m in range(NCORES * nh_a):
                            kg = (m // nh_a) * HL + heads_a[m % nh_a]
                            nc.tensor.matmul(
                                ps[:], wot[:, kg, oc * 128:(oc + 1) * 128],
                                agc_a[:, m, :],
                                start=(m == 0), stop=(m == NCORES * nh_a - 1))
                        oa = opool.tile([128, SC], dt.float32, tag="oa")
                        nc.vector.tensor_copy(oa[:], ps[:])
                        parts.append(oa)
                    agc_b, heads_b = loaded[1]
                    nh_b = len(heads_b)
                    for oc in range(4):
                        ps = pp3.tile([128, SC], dt.float32, tag="ps3")
                        for m in range(NCORES * nh_b):
                            kg = (m // nh_b) * HL + heads_b[m % nh_b]
                            nc.tensor.matmul(
                                ps[:], wot[:, kg, oc * 128:(oc + 1) * 128],
                                agc_b[:, m, :],
                                start=(m == 0), stop=(m == NCORES * nh_b - 1))
                        ot = opool.tile([128, SC], dt.float32, tag="ot")
                        nc.vector.tensor_add(ot[:], ps[:], parts[oc][:])
                        nc.sync.dma_start(out_d[oc * 128:(oc + 1) * 128, ssl], ot[:])

                def scores_exp(sc, h, ti):
                    # emits scores matmul + diag mask + exp; returns (pt, v0)
                    d_off = ti * 128 - sc * SC
                    v0 = max(d_off, 0)
                    vsl = slice(v0, SC)
                    qcl = slice(sc * SC + v0, (sc + 1) * SC)
                    st = stpool.tile([128, SC], dt.float32, tag="st")
                    nc.tensor.matmul(st[:, vsl],
                                     kt[:, ti * 128:(ti + 1) * 128],
                                     qt[h][:, qcl], start=True, stop=True)
                    if d_off >= 0:
                        nc.vector.tensor_add(st[:, d_off:d_off + 128],
                                             st[:, d_off:d_off + 128],
                                             dmask[:])
                    pt = ptpool.tile([128, SC], dt.bfloat16, tag="pt")
                    nc.scalar.activation(pt[:, vsl], st[:, vsl],
                                         mybir.ActivationFunctionType.Exp,
                                         scale=SCALE)
                    return pt, v0

                pending_ops = {}    # sc -> (sc, pieces) awaiting outproj
                # emission slots (sc, h) -> which chunk's outproj to emit;
                # placed so each gather has ample time to complete first
                emit_map = {(2, 0): 0, (2, 3): 1, (3, 3): 2}
                pending_ep = None   # deferred epilogue_b
                ag_a_cur = None
                LOOKAHEAD = 3
                for sc in range(NSC):
                    n_t = sc * 4 + 4
                    queue = [(h, ti) for h in range(HL) for ti in range(n_t)]
                    cache = {}
                    emitted = 0
                    pvrs = {}
                    for idx, (h, ti) in enumerate(queue):
                        while emitted <= min(idx + LOOKAHEAD, len(queue) - 1):
                            hq, tq = queue[emitted]
                            cache[(hq, tq)] = scores_exp(sc, hq, tq)
                            emitted += 1
                        if ti == 0:
                            pvrs[h] = (pvpool.tile([128, SC], dt.float32,
                                                   name="pv", tag="pv"),
                                       rspool.tile([1, SC], dt.float32,
                                                   name="rs", tag="rs"))
                        pv, rs = pvrs[h]
                        pt, v0 = cache.pop((h, ti))
                        vsl = slice(v0, SC)
                        nc.tensor.matmul(rs[:, vsl], onesc[:], pt[:, vsl],
                                         start=(ti == 0), stop=(ti == n_t - 1))
                        nc.tensor.matmul(pv[:, vsl], vv[:, ti, :], pt[:, vsl],
                                         start=(ti == 0), stop=(ti == n_t - 1))
                        if ti == n_t - 1:
                            ep = epilogue_a(sc, h, pv, rs)
                            if pending_ep is not None:
                                epilogue_b(*pending_ep)
                                pending_ep = None
                            pending_ep = ep
                            if h == 2:
                                # epilogue_b(h0) and (h1) are done by now
                                ag_a_cur = allgather_heads(sc, [0, 1])
                            opsc = emit_map.get((sc, h))
                            if opsc is not None and opsc in pending_ops:
                                outproj(*pending_ops.pop(opsc))
                    epilogue_b(*pending_ep)
                    pending_ep = None
                    ag_b_cur = allgather_heads(sc, [2, 3])
                    pending_ops[sc] = (sc, [ag_a_cur, ag_b_cur])
                for opsc in sorted(pending_ops):
                    outproj(*pending_ops[opsc])
    if not nc.is_finalized():
        nc.finalize()
    return nc


_CACHE = {}


def _get_nc():
    if "nc" not in _CACHE:
        _CACHE["nc"] = _build_nc()
    return _CACHE["nc"]


def _prep_in_maps(x, wq, wk, wv, wo, freqs_cos, freqs_sin):
    xt = np.ascontiguousarray(x.reshape(S, D).T).astype(BF)
    cosd = np.repeat(np.asarray(freqs_cos, np.float32).T, 2, axis=0).astype(BF)
    sind = np.repeat(np.asarray(freqs_sin, np.float32).T, 2, axis=0).astype(BF)
    swapt = np.zeros((HD, HD), np.float32)
    for i in range(HD // 2):
        swapt[2 * i + 1, 2 * i] = -1.0
        swapt[2 * i, 2 * i + 1] = 1.0
    swapt = swapt.astype(BF)
    ident = np.eye(HD, dtype=np.float32).astype(BF)
    t_idx = np.arange(128)[:, None]
    s_idx = np.arange(128)[None, :]
    dmask = np.where(s_idx >= t_idx, 0.0, NEG).astype(np.float32)
    onesc = np.ones((128, 1), np.float32).astype(BF)
    onesr = np.ones((1, 128), np.float32)

    wq = np.asarray(wq, np.float32)
    wk = np.asarray(wk, np.float32)
    wv = np.asarray(wv, np.float32)
    wo = np.asarray(wo, np.float32)

    in_maps = []
    for c in range(NCORES):
        qsl = slice(QW * c, QW * (c + 1))
        ksl = slice(HD * c, HD * (c + 1))
        in_maps.append({
            "xt": xt,
            "wqt": np.ascontiguousarray(wq[qsl].T).astype(BF),
            "wkt": np.ascontiguousarray(wk[ksl].T).astype(BF),
            "wvt": np.ascontiguousarray(wv[ksl].T).astype(BF),
            "wot": np.ascontiguousarray(wo[qsl].T).astype(BF),
            "cosd": cosd, "sind": sind, "swapt": swapt, "ident": ident,
            "dmask": dmask, "onesc": onesc, "onesr": onesr,
        })
    return in_maps


def _patch_ldw_opt():
    # walrus's LDWEIGHTS background-buffer optimization is disabled by
    # default in this flow; without it every matmul serializes a ~103ns
    # weight load. Flip the flag; the accuracy gate guards correctness.
    from concourse import bass_utils
    if getattr(bass_utils, "_ldw_patched", False):
        return
    orig = bass_utils.run_command

    def run_command_ldw(cmd, *a, **k):
        # (ldw-opt=true miscompiles: walrus visitInstLdweights error)
        return orig(cmd, *a, **k)

    bass_utils.run_command = run_command_ldw
    bass_utils._ldw_patched = True


def run(inputs, trace=False):
    _patch_ldw_opt()
    from concourse.bass_utils import run_bass_kernel_spmd
    nc = _get_nc()
    in_maps = _prep_in_maps(
        inputs["x"], inputs["wq"], inputs["wk"], inputs["wv"], inputs["wo"],
        inputs["freqs_cos"], inputs["freqs_sin"])
    res = run_bass_kernel_spmd(nc, in_maps, core_ids=list(range(NCORES)),
                               trace=trace)
    shards = [np.asarray(res.results[c]["out_t"], np.float32)
              for c in range(NCORES)]
    full = np.concatenate(shards, axis=0)          # [4096, 2048]
    out = np.ascontiguousarray(full.T)[None]       # [1, 2048, 4096]
    return out.astype(np.float32), res


def kernel(**inputs):
    out, _ = run(inputs, trace=False)
    return out
